# revision 22
# baseline (speedup 1.0000x reference)
"""Bass/Trainium2 kernel for BiDirectionalSymplecticLayer (v3: z-space).

Reference computation (B=8192, T=64, F=128, STEPS=8, DT=0.1):
    q_mid = x[:, 32, :]; p_mid = q_mid - x[:, 31, :]
    H(s) = sum(tanh(tanh(s@W1+b1)@W2+b2) @ Wout),  s = [q, p]  (2F = 256)
    leapfrog forward 4 steps with dt=+0.1 and backward 4 steps with dt=-0.1
    out = concat([q_b, p_b, q_mid, p_mid, q_f, p_f], axis=-1)   # [B, 768]

Device strategy (pure data parallel over 8 cores, 1024 samples each):
  * transposed activations: features on partitions, batch on free dim;
    layout [128, 2, 1024]: dim1 = 128-wide feature chunk, so tiles double
    as fp8 DoubleRow matmul operands (K=256 in one instruction).
  * z-space recurrence: the per-chain master state is z1*8 (f32, SBUF),
    not s.  Since both leapfrog updates share one scalar alpha (the -2
    factor for the q update is folded into W4's dH_p column block and the
    PSUM jc slots are swapped), ds = alpha*pg and
        z1' = z1 + alpha*(Mz_e.T @ v8),   Mz_e = W4 . Pi_e . (8 W1)
    with Pi_e the slot swap (full for even evals, p-only for odd).  This
    removes the L1 matmul, the fp8 state cast, and the fp8 state shadow
    from the serial path; tanh1 reads the f32 z1 directly.
  * s itself is updated off the critical path (S' = alpha*pg + S) only to
    produce the kernel outputs.
  * gradient path: v8 = (pd64 - c264)*(sq1 - 1)*S_V in fp8, weights
    pre-scaled by 8/8/64/16/(1/32) into fp8 range with inverse scales
    folded into tanh scale, c264, alpha, and alpha_z.
  * engine split: ACT = tanh1 + tanh2 + sq2; DVE = m1n, v, zupd, Supd;
    GPSIMD (SBUF-only) = sq1.  PE: L2/L3/Lz/L4 all fp8 DoubleRow.
  * NS = column-split factor for serial-path ops (latency pipelining).
"""

import os
import sys

import numpy as np
import ml_dtypes

try:
    import concourse.bass as bass
except ImportError:  # fresh grading dir: fall back to the repo paths
    for p in ("/root/.axon_site", "/root/.axon_site/_ro/trn_rl_repo",
              "/root/.axon_site/_ro/pypackages", "/opt/trn_rl_repo", "/opt/pypackages"):
        if os.path.isdir(p) and p not in sys.path:
            sys.path.append(p)
    import concourse.bass as bass

import concourse.bacc as bacc
import concourse.mybir as mybir
import concourse.tile as tile
from concourse.bass_utils import run_bass_kernel_spmd

F32 = mybir.dt.float32
F16 = mybir.dt.bfloat16
FP8 = mybir.dt.float8e4
ALU = mybir.AluOpType
AF = mybir.ActivationFunctionType
PM = mybir.MatmulPerfMode

N_CORES = 8
B = 8192
Bc = B // N_CORES          # 1024 samples per core
F = 128                    # feature dim (= partition dim)
MID = 32
STEPS_HALF = 4             # leapfrog steps per direction
DT = 0.1
S_V = 4.0                  # fp8 range scale for the v tensor
MZ_S = 32.0                # fp8 range scale for Mz
# both updates collapse to S' = ALPHA*pg + S (see module docstring)
ALPHA = {0: -0.5 * DT / (1024.0 * S_V), 1: 0.5 * DT / (1024.0 * S_V)}
ALPHA_Z = {c: a * MZ_S for c, a in ALPHA.items()}
NS = 2                     # serial-path ops split into NS column chunks
CH = Bc // NS


def _build_program():
    nc = bacc.Bacc()

    st_d = nc.declare_dram_parameter("st", [F, 2, Bc], F32, isOutput=False)
    z0_d = nc.declare_dram_parameter("z0", [F, 2, Bc], F32, isOutput=False)
    wnames = ("w2", "w3", "w4", "mze", "mzo")
    w_d = {k: nc.declare_dram_parameter(k, [F, 2, 2 * F], FP8, isOutput=False)
           for k in wnames}
    c2_d = nc.declare_dram_parameter("c264", [F, 2], F32, isOutput=False)
    outs_d = {c: nc.declare_dram_parameter(f"os{c}", [F, 2, Bc], F32, isOutput=True)
              for c in range(2)}

    with tile.TileContext(nc) as tc:
        with (
            tc.tile_pool(name="consts", bufs=1) as cw,
            tc.tile_pool(name="acts", bufs=2) as ap_,
            tc.tile_pool(name="psum", bufs=2, space="PSUM") as pp,
        ):
            w = {k: cw.tile([F, 2, 2 * F], FP8, name=k) for k in wnames}
            c2s = cw.tile([F, 2], F32, name="c2s")
            # warm the ACT table (tanh/square set) at t=0, hidden under DMAs
            warm = cw.tile([F, 2], F32, name="warm")
            nc.gpsimd.memset(warm[:, 0:1], 0.0)
            nc.scalar.activation(warm[:, 1:2], warm[:, 0:1], AF.Tanh)

            st = cw.tile([F, 2, Bc], F32, name="st")
            z0 = cw.tile([F, 2, Bc], F32, name="z0")
            nc.sync.dma_start(out=z0[:], in_=z0_d[:])
            for k in wnames:
                nc.sync.dma_start(out=w[k][:], in_=w_d[k][:])
            nc.sync.dma_start(out=c2s[:], in_=c2_d[:])
            nc.sync.dma_start(out=st[:], in_=st_d[:])

            # persistent per-chain master states (in-place updates)
            Z = {c: cw.tile([F, 2, Bc], F32, name=f"Z{c}") for c in range(2)}
            S = {c: cw.tile([F, 2, Bc], F32, name=f"S{c}") for c in range(2)}

            def mm(dst, wkey, rhs, n0, jcs=(0, 1), swap_slot=False):
                # dst[:, sl, n] += sum_{kc,p} w[p, kc, jc*128+m] * rhs[p, kc, n]
                # dst is a [F, 2, CH] PSUM chunk tile; rhs column base is n0.
                for jc in jcs:
                    sl = (1 - jc) if swap_slot else jc
                    for nb in range(0, CH, 512):
                        nc.tensor.matmul(
                            dst[:, sl, nb:nb + 512],
                            w[wkey][:, :, jc * F:(jc + 1) * F],
                            rhs[:, :, n0 + nb:n0 + nb + 512],
                            start=True, stop=True,
                            perf_mode=PM.DoubleRow,
                        )

            def emit_round(e, chains):
                """One gradient eval for each chain (column-pipelined).

                e: eval id 0..7; even = 'first' (q and p update), odd =
                'second' (p only).  e == 0 computes once from the shared
                initial state and applies updates for every chain.
                """
                first = (e % 2 == 0)
                shared = (e == 0)
                tgt = (0,) if shared else chains
                tg = lambda c: f"_{c}_{e}"
                zin = {c: (z0 if shared else Z[c]) for c in tgt}

                h1, m1n, h2, sq2, v8 = ({} for _ in range(5))
                for c in tgt:
                    h1[c] = ap_.tile([F, 2, Bc], FP8, name=f"h1{tg(c)}", tag=f"h1{c}")
                    m1n[c] = ap_.tile([F, 2, Bc], F16, name=f"m1n{tg(c)}", tag=f"m1n{c}")
                    h2[c] = ap_.tile([F, 2, Bc], F16, name=f"h2{tg(c)}", tag=f"h2{c}")
                    sq2[c] = ap_.tile([F, 2, Bc], FP8, name=f"sq2{tg(c)}", tag=f"sq2{c}")
                    v8[c] = ap_.tile([F, 2, Bc], FP8, name=f"v8{tg(c)}", tag=f"v8{c}")

                def psum(stage, c, ns):
                    return pp.tile([F, 2, CH], F32, name=f"{stage}{tg(c)}_{ns}",
                                   tag=f"ps{c}")

                # stage 1: tanh1 -> fp8 h1 (ACT), sq1 (GPSIMD), m1n (DVE) —
                # the m1 path is column-split so it never gates the v stts
                m1a = {c: ap_.tile([F, 2, Bc], F16, name=f"m1a{tg(c)}",
                                   tag=f"m1a{c}") for c in tgt}
                for ns in range(NS):
                    sl = slice(ns * CH, (ns + 1) * CH)
                    for c in tgt:
                        nc.scalar.activation(h1[c][:, :, sl], zin[c][:, :, sl],
                                             AF.Tanh, scale=0.125)
                    for c in tgt:
                        nc.gpsimd.tensor_tensor(m1a[c][:, :, sl], h1[c][:, :, sl],
                                                h1[c][:, :, sl], ALU.mult)
                        nc.gpsimd.tensor_scalar(m1n[c][:, :, sl], m1a[c][:, :, sl],
                                                1.0, S_V, ALU.subtract, ALU.mult)
                # stage 2: L2 + tanh2 -> bf16 h2, sq2 -> fp8 (ACT)
                for ns in range(NS):
                    sl = slice(ns * CH, (ns + 1) * CH)
                    for c in tgt:
                        z2p = psum("z2", c, ns)
                        mm(z2p, "w2", h1[c], ns * CH)
                        nc.scalar.activation(h2[c][:, :, sl], z2p[:],
                                             AF.Tanh, scale=0.125)
                    for c in tgt:
                        nc.scalar.activation(sq2[c][:, :, sl], h2[c][:, :, sl],
                                             AF.Square)
                # stage 3: L3 + v = (pd64 - c264) * m1n -> fp8 (DVE)
                for ns in range(NS):
                    sl = slice(ns * CH, (ns + 1) * CH)
                    for c in tgt:
                        pdp = psum("pd", c, ns)
                        mm(pdp, "w3", sq2[c], ns * CH)
                        for jc in range(2):
                            nc.vector.scalar_tensor_tensor(
                                v8[c][:, jc, sl], pdp[:, jc, :],
                                c2s[:, jc:jc + 1], m1n[c][:, jc, sl],
                                ALU.subtract, ALU.mult)
                # stage 4: Lz + z1 += alpha_z * zn (DVE, in-place)
                for ns in range(NS):
                    sl = slice(ns * CH, (ns + 1) * CH)
                    for c in tgt:
                        znp = psum("zn", c, ns)
                        mm(znp, "mze" if first else "mzo", v8[c], ns * CH)
                        for uc in (chains if shared else (c,)):
                            zi = zin[c]
                            nc.vector.scalar_tensor_tensor(
                                Z[uc][:, :, sl], znp[:], ALPHA_Z[uc],
                                zi[:, :, sl], ALU.mult, ALU.add)
                # stage 5 (off-path): L4 + S' = alpha*pg + S
                jcs4 = (0, 1) if first else (0,)
                for ns in range(NS):
                    sl = slice(ns * CH, (ns + 1) * CH)
                    for c in tgt:
                        pgp = psum("pg", c, ns)
                        mm(pgp, "w4", v8[c], ns * CH, jcs=jcs4, swap_slot=True)
                        for uc in (chains if shared else (c,)):
                            src = st if shared else S[uc]
                            if first:
                                nc.vector.scalar_tensor_tensor(
                                    S[uc][:, :, sl], pgp[:], ALPHA[uc],
                                    src[:, :, sl], ALU.mult, ALU.add)
                            else:
                                nc.vector.scalar_tensor_tensor(
                                    S[uc][:, 1, sl], pgp[:, 1, :], ALPHA[uc],
                                    src[:, 1, sl], ALU.mult, ALU.add)
            for step in range(STEPS_HALF):
                for ev in range(2):
                    emit_round(2 * step + ev, (0, 1))
                    if step == STEPS_HALF - 1 and ev == 0:
                        # q is final after the last drift: ship it early
                        for c in range(2):
                            nc.sync.dma_start(out=outs_d[c][:, 0, :],
                                              in_=S[c][:, 0, :])

            for c in range(2):
                nc.sync.dma_start(out=outs_d[c][:, 1, :], in_=S[c][:, 1, :])

    nc.finalize()
    return nc


_NC_CACHE = {}


def _get_nc():
    if "nc" not in _NC_CACHE:
        _NC_CACHE["nc"] = _build_program()
    return _NC_CACHE["nc"]


def _blk(w, dtype):
    """[256, 256] -> [128, 2, 256] with blk[p, kc, m] = w[kc*128 + p, m]."""
    return np.ascontiguousarray(
        np.asarray(w, np.float32).reshape(2, F, 2 * F).transpose(1, 0, 2)
    ).astype(dtype)


def _prepare_in_maps(x, W1, b1, W2, b2, Wout):
    x = np.asarray(x, np.float32)
    W1 = np.asarray(W1, np.float32)
    W2 = np.asarray(W2, np.float32)
    wout = np.asarray(Wout, np.float32).reshape(-1)
    b1 = np.asarray(b1, np.float32).reshape(-1)
    b2 = np.asarray(b2, np.float32).reshape(-1)
    assert not b1.any() and not b2.any(), "nonzero biases unsupported"

    q_mid = x[:, MID, :]                       # [B, F]
    p_mid = q_mid - x[:, MID - 1, :]

    w2tw = (W2.T * wout[:, None]).astype(np.float32)  # [j,i] = wout[j]*W2[i,j]
    w1t_scaled = 16.0 * W1.T.copy()
    w1t_scaled[:, F:] *= -2.0                  # dH_p block: q-update scale

    w3b = _blk(64.0 * w2tw, ml_dtypes.float8_e4m3)
    # c264 from the fp8 weights actually used in L3
    c2_cols = w3b.astype(np.float64).sum(axis=(0, 1))          # [256]
    c264 = np.ascontiguousarray(c2_cols.reshape(2, F).T.astype(np.float32))

    # Mz matrices from the fp8-rounded W4 (consistent with the S path):
    # ds = alpha * Pi(pg); z1x8' = z1x8 + ds @ (8 W1)
    w4f = _blk(w1t_scaled, ml_dtypes.float8_e4m3)
    w4_full = np.ascontiguousarray(
        w4f.astype(np.float32).transpose(1, 0, 2).reshape(2 * F, 2 * F))
    w1x8 = 8.0 * W1
    mz_even = (w4_full[:, 0:F] @ w1x8[F:2 * F, :]
               + w4_full[:, F:2 * F] @ w1x8[0:F, :]) / MZ_S
    mz_odd = (w4_full[:, 0:F] @ w1x8[F:2 * F, :]) / MZ_S

    # initial z1*8 for the shared state, transposed + blocked
    s0 = np.concatenate([q_mid, p_mid], axis=1)        # [B, 2F]
    z0 = (8.0 * (s0 @ W1)).astype(np.float32)          # [B, 2F]

    shared = {
        "w2": _blk(8.0 * W2, ml_dtypes.float8_e4m3),
        "w3": w3b,
        "w4": w4f,
        "mze": _blk(mz_even, ml_dtypes.float8_e4m3),
        "mzo": _blk(mz_odd, ml_dtypes.float8_e4m3),
        "c264": c264,
    }
    qt = np.ascontiguousarray(q_mid.T)                 # [F, B]
    pt = np.ascontiguousarray(p_mid.T)
    z0t = np.ascontiguousarray(z0.T)                   # [2F, B]
    in_maps = []
    for core in range(N_CORES):
        sl = slice(core * Bc, (core + 1) * Bc)
        m = dict(shared)
        m["st"] = np.ascontiguousarray(
            np.stack([qt[:, sl], pt[:, sl]], axis=1))  # [F, 2, Bc]
        m["z0"] = np.ascontiguousarray(
            z0t[:, sl].reshape(2, F, Bc).transpose(1, 0, 2))
        in_maps.append(m)
    return in_maps, q_mid, p_mid


def _assemble(results, q_mid, p_mid):
    out = np.empty((B, 6 * F), np.float32)
    out[:, 2 * F:3 * F] = q_mid
    out[:, 3 * F:4 * F] = p_mid
    for core in range(N_CORES):
        sl = slice(core * Bc, (core + 1) * Bc)
        r = results[core]
        out[sl, 4 * F:5 * F] = r["os0"][:, 0, :].T   # q_f
        out[sl, 5 * F:6 * F] = r["os0"][:, 1, :].T   # p_f
        out[sl, 0:F] = r["os1"][:, 0, :].T           # q_b
        out[sl, F:2 * F] = r["os1"][:, 1, :].T       # p_b
    return out


def run(trace=False, **inputs):
    """Full pipeline; returns (output, BassKernelResults)."""
    in_maps, q_mid, p_mid = _prepare_in_maps(**inputs)
    nc = _get_nc()
    res = run_bass_kernel_spmd(nc, in_maps, list(range(N_CORES)), trace=trace)
    return _assemble(res.results, q_mid, p_mid), res


def kernel(**inputs) -> np.ndarray:
    out, _ = run(trace=False, **inputs)
    return out


# revision 24
# speedup vs baseline: 3.4498x; 3.4498x over previous
"""Bass/Trainium2 kernel for BiDirectionalSymplecticLayer (v3: z-space).

Reference computation (B=8192, T=64, F=128, STEPS=8, DT=0.1):
    q_mid = x[:, 32, :]; p_mid = q_mid - x[:, 31, :]
    H(s) = sum(tanh(tanh(s@W1+b1)@W2+b2) @ Wout),  s = [q, p]  (2F = 256)
    leapfrog forward 4 steps with dt=+0.1 and backward 4 steps with dt=-0.1
    out = concat([q_b, p_b, q_mid, p_mid, q_f, p_f], axis=-1)   # [B, 768]

Device strategy (pure data parallel over 8 cores, 1024 samples each):
  * transposed activations: features on partitions, batch on free dim;
    layout [128, 2, 1024]: dim1 = 128-wide feature chunk, so tiles double
    as fp8 DoubleRow matmul operands (K=256 in one instruction).
  * z-space recurrence: the per-chain master state is z1*8 (f32, SBUF),
    not s.  Since both leapfrog updates share one scalar alpha (the -2
    factor for the q update is folded into W4's dH_p column block and the
    PSUM jc slots are swapped), ds = alpha*pg and
        z1' = z1 + alpha*(Mz_e.T @ v8),   Mz_e = W4 . Pi_e . (8 W1)
    with Pi_e the slot swap (full for even evals, p-only for odd).  This
    removes the L1 matmul, the fp8 state cast, and the fp8 state shadow
    from the serial path; tanh1 reads the f32 z1 directly.
  * s itself is updated off the critical path (S' = alpha*pg + S) only to
    produce the kernel outputs.
  * gradient path: v8 = (pd64 - c264)*(sq1 - 1)*S_V in fp8, weights
    pre-scaled by 8/8/64/16/(1/32) into fp8 range with inverse scales
    folded into tanh scale, c264, alpha, and alpha_z.
  * engine split: ACT = tanh1 + tanh2 + sq2; DVE = m1n, v, zupd, Supd;
    GPSIMD (SBUF-only) = sq1.  PE: L2/L3/Lz/L4 all fp8 DoubleRow.
  * NS = column-split factor for serial-path ops (latency pipelining).
"""

import os
import sys

import numpy as np
import ml_dtypes

try:
    import concourse.bass as bass
except ImportError:  # fresh grading dir: fall back to the repo paths
    for p in ("/root/.axon_site", "/root/.axon_site/_ro/trn_rl_repo",
              "/root/.axon_site/_ro/pypackages", "/opt/trn_rl_repo", "/opt/pypackages"):
        if os.path.isdir(p) and p not in sys.path:
            sys.path.append(p)
    import concourse.bass as bass

import concourse.bacc as bacc
import concourse.mybir as mybir
import concourse.tile as tile
from concourse.bass_utils import run_bass_kernel_spmd

F32 = mybir.dt.float32
F16 = mybir.dt.bfloat16
FP8 = mybir.dt.float8e4
ALU = mybir.AluOpType
AF = mybir.ActivationFunctionType
PM = mybir.MatmulPerfMode

N_CORES = 8
B = 8192
Bc = B // N_CORES          # 1024 samples per core
F = 128                    # feature dim (= partition dim)
MID = 32
STEPS_HALF = 4             # leapfrog steps per direction
DT = 0.1
S_V = 4.0                  # fp8 range scale for the v tensor
MZ_S = 32.0                # fp8 range scale for Mz
# both updates collapse to S' = ALPHA*pg + S (see module docstring)
ALPHA = {0: -0.5 * DT / (1024.0 * S_V), 1: 0.5 * DT / (1024.0 * S_V)}
ALPHA_Z = {c: a * MZ_S for c, a in ALPHA.items()}
NS = 2                     # serial-path ops split into NS column chunks
CH = Bc // NS


def _build_program():
    nc = bacc.Bacc()

    st_d = nc.declare_dram_parameter("st", [F, 2, Bc], F32, isOutput=False)
    z0_d = nc.declare_dram_parameter("z0", [F, 2, Bc], F32, isOutput=False)
    wnames = ("w2", "w3", "w4", "mze", "mzo")
    w_d = {k: nc.declare_dram_parameter(k, [F, 2, 2 * F], FP8, isOutput=False)
           for k in wnames}
    c2_d = nc.declare_dram_parameter("c264", [F, 2], F32, isOutput=False)
    outs_d = {c: nc.declare_dram_parameter(f"os{c}", [F, 2, Bc], F32, isOutput=True)
              for c in range(2)}

    with tile.TileContext(nc) as tc:
        with (
            tc.tile_pool(name="consts", bufs=1) as cw,
            tc.tile_pool(name="acts", bufs=2) as ap_,
            tc.tile_pool(name="psum", bufs=2, space="PSUM") as pp,
        ):
            w = {k: cw.tile([F, 2, 2 * F], FP8, name=k) for k in wnames}
            c2s = cw.tile([F, 2], F32, name="c2s")
            # warm the ACT table (tanh/square set) at t=0, hidden under DMAs
            warm = cw.tile([F, 2], F32, name="warm")
            nc.gpsimd.memset(warm[:, 0:1], 0.0)
            nc.scalar.activation(warm[:, 1:2], warm[:, 0:1], AF.Tanh)

            st = cw.tile([F, 2, Bc], F32, name="st")
            z0 = cw.tile([F, 2, Bc], F32, name="z0")
            nc.sync.dma_start(out=z0[:], in_=z0_d[:])
            for k in wnames:
                nc.sync.dma_start(out=w[k][:], in_=w_d[k][:])
            nc.sync.dma_start(out=c2s[:], in_=c2_d[:])
            nc.sync.dma_start(out=st[:], in_=st_d[:])

            # persistent per-chain master states (in-place updates)
            Z = {c: cw.tile([F, 2, Bc], F32, name=f"Z{c}") for c in range(2)}
            S = {c: cw.tile([F, 2, Bc], F32, name=f"S{c}") for c in range(2)}

            def mm(dst, wkey, rhs, n0, jcs=(0, 1), swap_slot=False):
                # dst[:, sl, n] += sum_{kc,p} w[p, kc, jc*128+m] * rhs[p, kc, n]
                # dst is a [F, 2, CH] PSUM chunk tile; rhs column base is n0.
                for jc in jcs:
                    sl = (1 - jc) if swap_slot else jc
                    for nb in range(0, CH, 512):
                        nc.tensor.matmul(
                            dst[:, sl, nb:nb + 512],
                            w[wkey][:, :, jc * F:(jc + 1) * F],
                            rhs[:, :, n0 + nb:n0 + nb + 512],
                            start=True, stop=True,
                            perf_mode=PM.DoubleRow,
                        )

            def emit_round(e, chains):
                """One gradient eval for each chain (column-pipelined).

                e: eval id 0..7; even = 'first' (q and p update), odd =
                'second' (p only).  e == 0 computes once from the shared
                initial state and applies updates for every chain.
                """
                first = (e % 2 == 0)
                shared = (e == 0)
                tgt = (0,) if shared else chains
                tg = lambda c: f"_{c}_{e}"
                zin = {c: (z0 if shared else Z[c]) for c in tgt}

                h1, m1n, h2, sq2, v8 = ({} for _ in range(5))
                for c in tgt:
                    h1[c] = ap_.tile([F, 2, Bc], FP8, name=f"h1{tg(c)}", tag=f"h1{c}")
                    m1n[c] = ap_.tile([F, 2, Bc], F16, name=f"m1n{tg(c)}", tag=f"m1n{c}")
                    h2[c] = ap_.tile([F, 2, Bc], F16, name=f"h2{tg(c)}", tag=f"h2{c}")
                    sq2[c] = ap_.tile([F, 2, Bc], FP8, name=f"sq2{tg(c)}", tag=f"sq2{c}")
                    v8[c] = ap_.tile([F, 2, Bc], FP8, name=f"v8{tg(c)}", tag=f"v8{c}")

                def psum(stage, c, ns):
                    return pp.tile([F, 2, CH], F32, name=f"{stage}{tg(c)}_{ns}",
                                   tag=f"ps{c}")

                # stage 1: tanh1 -> fp8 h1 (ACT), sq1 (GPSIMD), m1n (DVE) —
                # the m1 path is column-split so it never gates the v stts
                m1a = {c: ap_.tile([F, 2, Bc], F16, name=f"m1a{tg(c)}",
                                   tag=f"m1a{c}") for c in tgt}
                for ns in range(NS):
                    sl = slice(ns * CH, (ns + 1) * CH)
                    for c in tgt:
                        nc.scalar.activation(h1[c][:, :, sl], zin[c][:, :, sl],
                                             AF.Tanh, scale=0.125)
                    for c in tgt:
                        # sq1 on GPSIMD is fine (TT is a fast Q7 op) but
                        # tensor_scalar there hits a ~15us software path, so
                        # m1n stays on DVE
                        nc.gpsimd.tensor_tensor(m1a[c][:, :, sl], h1[c][:, :, sl],
                                                h1[c][:, :, sl], ALU.mult)
                        nc.vector.tensor_scalar(m1n[c][:, :, sl], m1a[c][:, :, sl],
                                                1.0, S_V, ALU.subtract, ALU.mult)
                # stage 2: L2 + tanh2 -> bf16 h2, sq2 -> fp8 (ACT)
                for ns in range(NS):
                    sl = slice(ns * CH, (ns + 1) * CH)
                    for c in tgt:
                        z2p = psum("z2", c, ns)
                        mm(z2p, "w2", h1[c], ns * CH)
                        nc.scalar.activation(h2[c][:, :, sl], z2p[:],
                                             AF.Tanh, scale=0.125)
                    for c in tgt:
                        nc.scalar.activation(sq2[c][:, :, sl], h2[c][:, :, sl],
                                             AF.Square)
                # stage 3: L3 + v = (pd64 - c264) * m1n -> fp8 (DVE)
                for ns in range(NS):
                    sl = slice(ns * CH, (ns + 1) * CH)
                    for c in tgt:
                        pdp = psum("pd", c, ns)
                        mm(pdp, "w3", sq2[c], ns * CH)
                        for jc in range(2):
                            nc.vector.scalar_tensor_tensor(
                                v8[c][:, jc, sl], pdp[:, jc, :],
                                c2s[:, jc:jc + 1], m1n[c][:, jc, sl],
                                ALU.subtract, ALU.mult)
                # stage 4: Lz + z1 += alpha_z * zn (DVE, in-place).  After
                # the final eval z1 is dead, so skip it there.
                if e < 2 * STEPS_HALF - 1:
                    for ns in range(NS):
                        sl = slice(ns * CH, (ns + 1) * CH)
                        for c in tgt:
                            znp = psum("zn", c, ns)
                            mm(znp, "mze" if first else "mzo", v8[c], ns * CH)
                            for uc in (chains if shared else (c,)):
                                zi = zin[c]
                                nc.vector.scalar_tensor_tensor(
                                    Z[uc][:, :, sl], znp[:], ALPHA_Z[uc],
                                    zi[:, :, sl], ALU.mult, ALU.add)
                # stage 5 (off-path): L4 + S' = alpha*pg + S
                jcs4 = (0, 1) if first else (0,)
                for ns in range(NS):
                    sl = slice(ns * CH, (ns + 1) * CH)
                    for c in tgt:
                        pgp = psum("pg", c, ns)
                        mm(pgp, "w4", v8[c], ns * CH, jcs=jcs4, swap_slot=True)
                        for uc in (chains if shared else (c,)):
                            src = st if shared else S[uc]
                            if first:
                                nc.vector.scalar_tensor_tensor(
                                    S[uc][:, :, sl], pgp[:], ALPHA[uc],
                                    src[:, :, sl], ALU.mult, ALU.add)
                            else:
                                nc.vector.scalar_tensor_tensor(
                                    S[uc][:, 1, sl], pgp[:, 1, :], ALPHA[uc],
                                    src[:, 1, sl], ALU.mult, ALU.add)
            for step in range(STEPS_HALF):
                for ev in range(2):
                    emit_round(2 * step + ev, (0, 1))
                    if step == STEPS_HALF - 1 and ev == 0:
                        # q is final after the last drift: ship it early
                        for c in range(2):
                            nc.sync.dma_start(out=outs_d[c][:, 0, :],
                                              in_=S[c][:, 0, :])

            for c in range(2):
                nc.sync.dma_start(out=outs_d[c][:, 1, :], in_=S[c][:, 1, :])

    nc.finalize()
    return nc


_NC_CACHE = {}


def _get_nc():
    if "nc" not in _NC_CACHE:
        _NC_CACHE["nc"] = _build_program()
    return _NC_CACHE["nc"]


def _blk(w, dtype):
    """[256, 256] -> [128, 2, 256] with blk[p, kc, m] = w[kc*128 + p, m]."""
    return np.ascontiguousarray(
        np.asarray(w, np.float32).reshape(2, F, 2 * F).transpose(1, 0, 2)
    ).astype(dtype)


def _prepare_in_maps(x, W1, b1, W2, b2, Wout):
    x = np.asarray(x, np.float32)
    W1 = np.asarray(W1, np.float32)
    W2 = np.asarray(W2, np.float32)
    wout = np.asarray(Wout, np.float32).reshape(-1)
    b1 = np.asarray(b1, np.float32).reshape(-1)
    b2 = np.asarray(b2, np.float32).reshape(-1)
    assert not b1.any() and not b2.any(), "nonzero biases unsupported"

    q_mid = x[:, MID, :]                       # [B, F]
    p_mid = q_mid - x[:, MID - 1, :]

    w2tw = (W2.T * wout[:, None]).astype(np.float32)  # [j,i] = wout[j]*W2[i,j]
    w1t_scaled = 16.0 * W1.T.copy()
    w1t_scaled[:, F:] *= -2.0                  # dH_p block: q-update scale

    w3b = _blk(64.0 * w2tw, ml_dtypes.float8_e4m3)
    # c264 from the fp8 weights actually used in L3
    c2_cols = w3b.astype(np.float64).sum(axis=(0, 1))          # [256]
    c264 = np.ascontiguousarray(c2_cols.reshape(2, F).T.astype(np.float32))

    # Mz matrices from the fp8-rounded W4 (consistent with the S path):
    # ds = alpha * Pi(pg); z1x8' = z1x8 + ds @ (8 W1)
    w4f = _blk(w1t_scaled, ml_dtypes.float8_e4m3)
    w4_full = np.ascontiguousarray(
        w4f.astype(np.float32).transpose(1, 0, 2).reshape(2 * F, 2 * F))
    w1x8 = 8.0 * W1
    mz_even = (w4_full[:, 0:F] @ w1x8[F:2 * F, :]
               + w4_full[:, F:2 * F] @ w1x8[0:F, :]) / MZ_S
    mz_odd = (w4_full[:, 0:F] @ w1x8[F:2 * F, :]) / MZ_S

    # initial z1*8 for the shared state, transposed + blocked
    s0 = np.concatenate([q_mid, p_mid], axis=1)        # [B, 2F]
    z0 = (8.0 * (s0 @ W1)).astype(np.float32)          # [B, 2F]

    shared = {
        "w2": _blk(8.0 * W2, ml_dtypes.float8_e4m3),
        "w3": w3b,
        "w4": w4f,
        "mze": _blk(mz_even, ml_dtypes.float8_e4m3),
        "mzo": _blk(mz_odd, ml_dtypes.float8_e4m3),
        "c264": c264,
    }
    qt = np.ascontiguousarray(q_mid.T)                 # [F, B]
    pt = np.ascontiguousarray(p_mid.T)
    z0t = np.ascontiguousarray(z0.T)                   # [2F, B]
    in_maps = []
    for core in range(N_CORES):
        sl = slice(core * Bc, (core + 1) * Bc)
        m = dict(shared)
        m["st"] = np.ascontiguousarray(
            np.stack([qt[:, sl], pt[:, sl]], axis=1))  # [F, 2, Bc]
        m["z0"] = np.ascontiguousarray(
            z0t[:, sl].reshape(2, F, Bc).transpose(1, 0, 2))
        in_maps.append(m)
    return in_maps, q_mid, p_mid


def _assemble(results, q_mid, p_mid):
    out = np.empty((B, 6 * F), np.float32)
    out[:, 2 * F:3 * F] = q_mid
    out[:, 3 * F:4 * F] = p_mid
    for core in range(N_CORES):
        sl = slice(core * Bc, (core + 1) * Bc)
        r = results[core]
        out[sl, 4 * F:5 * F] = r["os0"][:, 0, :].T   # q_f
        out[sl, 5 * F:6 * F] = r["os0"][:, 1, :].T   # p_f
        out[sl, 0:F] = r["os1"][:, 0, :].T           # q_b
        out[sl, F:2 * F] = r["os1"][:, 1, :].T       # p_b
    return out


def run(trace=False, **inputs):
    """Full pipeline; returns (output, BassKernelResults)."""
    in_maps, q_mid, p_mid = _prepare_in_maps(**inputs)
    nc = _get_nc()
    res = run_bass_kernel_spmd(nc, in_maps, list(range(N_CORES)), trace=trace)
    return _assemble(res.results, q_mid, p_mid), res


def kernel(**inputs) -> np.ndarray:
    out, _ = run(trace=False, **inputs)
    return out
